# revision 7
# baseline (speedup 1.0000x reference)
"""Dcls1d (Gaussian-parameterized dilated conv1d) Trainium2 Bass kernel.

Math (reference):
    W   = weight * sign                               (O, I, C)
    Pc  = P[0] + KD//2 ; S = |SIG[0]| + 0.27          (O, I, C)
    X_d = exp(-0.5 * ((d - Pc)/S)^2)                  d = 0..KD-1
    K   = sum_c X_d * W / (sum_d' X_d' + 1e-7)        (O, I, KD)
    out = conv1d(x, K, VALID)                         (B, O, L-KD+1)

Distribution over 8 NeuronCores:
  - kernel construction: out-channel-sharded (32 out-channels per core)
  - AllGather of the (O, I, KD) kernel (as matmul-ready lhsT layout)
  - conv: batch-sharded (4 batches per core), 50 accumulating PE matmuls
    per output tile in float32r (single-pass fp22).

Construction layout per core: partitions p = i mod 128, free = (j, c) where
j = ih*32 + o_loc indexes the 64 (o_loc, i-half) tiles. The per-d Gaussian
argument is one fused scalar_tensor_tensor: m = (Pc - d) * R, whose sign is
killed by Square. X is stored bf16; Z is one strided reduce over d.
"""

import os

import numpy as np

import concourse.bass as bass
import concourse.mybir as mybir
import concourse.tile as tile
from concourse import bacc
from concourse.bass_utils import run_bass_kernel_spmd

F32 = mybir.dt.float32
F32R = mybir.dt.float32r
BF16 = mybir.dt.bfloat16
AF = mybir.ActivationFunctionType
ALU = mybir.AluOpType

B, OC, IC, L = 32, 256, 256, 1024
KC, KD = 26, 25
NC = 8
O_SH = OC // NC          # 32 out-channels per core
NIB = IC // 128          # 2 i-blocks
NT = O_SH * NIB          # 64 construction tiles per core
FB = NT * KC             # 1664 free width of the "small" tensors
B_SH = B // NC           # 4 batches per core
TO = L - KD + 1          # 1000 output positions
TC = 500                 # conv t-chunk (PSUM bank = 512 fp32 max)
NTC = TO // TC           # 2
NOB = OC // 128          # 2 out-channel blocks


def build_module():
    nc = bacc.Bacc("TRN2", num_devices=NC)

    p_in = nc.dram_tensor("p_in", [128, FB], F32, kind="ExternalInput")
    sig_in = nc.dram_tensor("sig_in", [128, FB], F32, kind="ExternalInput")
    w_in = nc.dram_tensor("w_in", [128, FB], F32, kind="ExternalInput")
    sgn_in = nc.dram_tensor("sgn_in", [128, FB], F32, kind="ExternalInput")
    x_in = nc.dram_tensor("x_in", [B_SH, NIB, 128, L], F32R, kind="ExternalInput")
    out_t = nc.dram_tensor("out", [B_SH, OC, TO], F32, kind="ExternalOutput")

    kshard = nc.dram_tensor("kshard", [KD, NIB, 128, O_SH], F32R)
    kgath = nc.dram_tensor(
        "kgath", [NC, KD, NIB, 128, O_SH], F32R, addr_space="Shared"
    )

    with tile.TileContext(nc) as tc:
        # ---------------- phase 1: kernel construction ----------------
        with tc.tile_pool(name="smalls", bufs=1) as smalls, \
             tc.tile_pool(name="xbig", bufs=1) as xbig, \
             tc.tile_pool(name="dtmp", bufs=2) as dtmp:
            p_sb = smalls.tile([128, FB], F32)
            sig_sb = smalls.tile([128, FB], F32)
            w_sb = smalls.tile([128, FB], F32)
            sgn_sb = smalls.tile([128, FB], F32)
            nc.sync.dma_start(p_sb[:], p_in[:])
            nc.sync.dma_start(sig_sb[:], sig_in[:])
            nc.sync.dma_start(w_sb[:], w_in[:])
            nc.sync.dma_start(sgn_sb[:], sgn_in[:])

            # Pc = P + KD//2 (in place); R = 1/(|SIG| + 0.27) (in place)
            pc_sb = p_sb
            nc.vector.tensor_scalar_add(pc_sb[:], p_sb[:], float(KD // 2))
            nc.scalar.activation(sig_sb[:], sig_sb[:], AF.Abs)
            nc.vector.tensor_scalar_add(sig_sb[:], sig_sb[:], 0.27)
            r_sb = sig_sb
            nc.vector.reciprocal(r_sb[:], sig_sb[:])

            # Wp = weight * sign (in place into w_sb; Wn needs Z)
            wp_sb = w_sb
            nc.vector.tensor_mul(wp_sb[:], w_sb[:], sgn_sb[:])

            # X_d = exp(-0.5 * ((Pc - d) * R)^2), stored bf16, free = (d, j, c)
            x_all = xbig.tile([128, KD * FB], BF16)
            for d in range(KD):
                m = dtmp.tile([128, FB], F32, tag="m")
                nc.vector.scalar_tensor_tensor(
                    m[:], pc_sb[:], float(d), r_sb[:],
                    op0=ALU.subtract, op1=ALU.mult,
                )
                sq = dtmp.tile([128, FB], F32, tag="sq")
                nc.scalar.activation(sq[:], m[:], AF.Square)
                nc.scalar.activation(
                    x_all[:, d * FB:(d + 1) * FB], sq[:], AF.Exp, scale=-0.5
                )

            # Z = sum_d X_d : one strided reduce (innermost axis = d)
            z_sb = smalls.tile([128, FB], F32)
            x_dz = x_all.rearrange("p (d f) -> p f d", d=KD)
            nc.vector.reduce_sum(z_sb[:], x_dz, axis=mybir.AxisListType.X)

            # Wn = Wp / (Z + 1e-7), cast bf16 for the 2x-mode multiply
            nc.vector.tensor_scalar_add(z_sb[:], z_sb[:], 1e-7)
            nc.vector.reciprocal(z_sb[:], z_sb[:])
            wn_sb = sgn_sb
            nc.vector.tensor_mul(wn_sb[:], wp_sb[:], z_sb[:])
            wn16 = smalls.tile([128, FB], BF16)
            nc.vector.tensor_copy(wn16[:], wn_sb[:])

            # K[p, (d, j)] = sum_c X_d[p, (j, c)] * Wn[p, (j, c)]
            k_sb = smalls.tile([128, KD * NT], F32R)
            with nc.allow_low_precision("float32r is fp32-width"):
                for d in range(KD):
                    y = dtmp.tile([128, FB], BF16, tag="y")
                    nc.vector.tensor_mul(
                        y[:], x_all[:, d * FB:(d + 1) * FB], wn16[:]
                    )
                    y3 = y.rearrange("p (j c) -> p j c", c=KC)
                    nc.vector.reduce_sum(
                        k_sb[:, d * NT:(d + 1) * NT], y3,
                        axis=mybir.AxisListType.X,
                    )

            # store shard: k_sb [p, (d, ih, ol)] -> kshard (d, ih, p, ol)
            ksb_v = k_sb.rearrange("p (d ih ol) -> p d ih ol", d=KD, ih=NIB)
            kout_v = kshard[:].rearrange("d ih p ol -> p d ih ol")
            nc.sync.dma_start(kout_v, ksb_v)

        # ---------------- phase 2: all-gather ----------------
        nc.gpsimd.collective_compute(
            "AllGather",
            ALU.bypass,
            replica_groups=[list(range(NC))],
            ins=[kshard[:]],
            outs=[kgath[:]],
        )

        # ---------------- phase 3: conv ----------------
        with tc.tile_pool(name="kw", bufs=1) as kw, \
             tc.tile_pool(name="xp", bufs=1) as xp, \
             tc.tile_pool(name="ps", bufs=4, space="PSUM") as ps, \
             tc.tile_pool(name="ob", bufs=3) as obp:
            lhsT = {}
            for d in range(KD):
                for ih in range(NIB):
                    t = kw.tile([128, OC], F32R, tag=f"k{d}_{ih}")
                    src = kgath[:, d, ih, :, :].rearrange("core p ol -> p core ol")
                    nc.sync.dma_start(t[:].rearrange("p (core ol) -> p core ol", core=NC), src)
                    lhsT[(d, ih)] = t

            x_sb = {}
            for b in range(B_SH):
                for ih in range(NIB):
                    t = xp.tile([128, L], F32R, tag=f"x{b}_{ih}")
                    nc.sync.dma_start(t[:], x_in[b, ih, :, :])
                    x_sb[(b, ih)] = t

            for b in range(B_SH):
                for ob in range(NOB):
                    for tck in range(NTC):
                        acc = ps.tile([128, TC], F32, tag="acc")
                        n = 0
                        for ih in range(NIB):
                            for d in range(KD):
                                nc.tensor.matmul(
                                    acc[:],
                                    lhsT[(d, ih)][:, ob * 128:(ob + 1) * 128],
                                    x_sb[(b, ih)][:, tck * TC + d: tck * TC + d + TC],
                                    start=(n == 0),
                                    stop=(n == NIB * KD - 1),
                                )
                                n += 1
                        o_sb = obp.tile([128, TC], F32, tag="osb")
                        nc.scalar.copy(o_sb[:], acc[:])
                        nc.sync.dma_start(
                            out_t[b, ob * 128:(ob + 1) * 128, tck * TC:(tck + 1) * TC],
                            o_sb[:],
                        )

    nc.compile()
    return nc


def make_in_maps(x, weight, sign, P, SIG):
    """Slice/pack full inputs into per-core input maps (pure layout work)."""
    x = np.ascontiguousarray(x, dtype=np.float32)
    in_maps = []
    for c in range(NC):
        osl = slice(O_SH * c, O_SH * c + O_SH)

        def pack(a):
            # (O_SH, IC, KC) -> [p = i mod 128, (j = ih*32 + o_loc, c)]
            a = np.asarray(a, dtype=np.float32).reshape(O_SH, NIB, 128, KC)
            a = a.transpose(1, 0, 2, 3).reshape(NT, 128, KC)
            return np.ascontiguousarray(
                a.transpose(1, 0, 2).reshape(128, NT * KC)
            )

        in_maps.append({
            "p_in": pack(P[0][osl]),
            "sig_in": pack(SIG[0][osl]),
            "w_in": pack(weight[osl]),
            "sgn_in": pack(sign[osl]),
            "x_in": np.ascontiguousarray(
                x[B_SH * c: B_SH * c + B_SH].reshape(B_SH, NIB, 128, L)
            ),
        })
    return in_maps


_CACHED = {}


def kernel(x, weight, sign, P, SIG, trace=False):
    if "nc" not in _CACHED:
        _CACHED["nc"] = build_module()
    nc = _CACHED["nc"]
    in_maps = make_in_maps(x, weight, sign, P, SIG)
    res = run_bass_kernel_spmd(
        nc, in_maps, core_ids=list(range(NC)), trace=trace,
    )
    out = np.concatenate([r["out"] for r in res.results], axis=0)
    if trace:
        _CACHED["last_result"] = res
    return out


# revision 15
# speedup vs baseline: 1.1664x; 1.1664x over previous
"""Dcls1d (Gaussian-parameterized dilated conv1d) Trainium2 Bass kernel.

Math (reference):
    W   = weight * sign                               (O, I, C)
    Pc  = P[0] + KD//2 ; S = |SIG[0]| + 0.27          (O, I, C)
    X_d = exp(-0.5 * ((d - Pc)/S)^2)                  d = 0..KD-1
    K   = sum_c X_d * W / (sum_d' X_d' + 1e-7)        (O, I, KD)
    out = conv1d(x, K, VALID)                         (B, O, L-KD+1)

Distribution over 8 NeuronCores:
  - kernel construction: out-channel-sharded (32 out-channels per core)
  - AllGather of the (O, I, KD) kernel (as matmul-ready lhsT layout)
  - conv: batch-sharded (4 batches per core), 50 accumulating PE matmuls
    per output tile in float32r (single-pass fp22).

Construction layout per core: partitions p = i mod 128, free = (j, c) where
j = ih*32 + o_loc indexes the 64 (o_loc, i-half) tiles. The per-d Gaussian
argument is one fused scalar_tensor_tensor: m = (Pc - d) * R, whose sign is
killed by Square. X is stored bf16; Z is one strided reduce over d.
"""

import os

import numpy as np

import concourse.bass as bass
import concourse.mybir as mybir
import concourse.tile as tile
from concourse import bacc
from concourse.bass_utils import run_bass_kernel_spmd

F32 = mybir.dt.float32
F32R = mybir.dt.float32r
BF16 = mybir.dt.bfloat16
AF = mybir.ActivationFunctionType
ALU = mybir.AluOpType

B, OC, IC, L = 32, 256, 256, 1024
KC, KD = 26, 25
NC = 8
O_SH = OC // NC          # 32 out-channels per core
NIB = IC // 128          # 2 i-blocks
NT = O_SH * NIB          # 64 construction tiles per core
FB = NT * KC             # 1664 free width of the "small" tensors
B_SH = B // NC           # 4 batches per core
TO = L - KD + 1          # 1000 output positions
TC = 500                 # conv t-chunk (PSUM bank = 512 fp32 max)
NTC = TO // TC           # 2
NOB = OC // 128          # 2 out-channel blocks


def build_module():
    nc = bacc.Bacc("TRN2", num_devices=NC)

    p_in = nc.dram_tensor("p_in", [128, FB], F32, kind="ExternalInput")
    sig_in = nc.dram_tensor("sig_in", [128, FB], F32, kind="ExternalInput")
    w_in = nc.dram_tensor("w_in", [128, FB], F32, kind="ExternalInput")
    sgn_in = nc.dram_tensor("sgn_in", [128, FB], F32, kind="ExternalInput")
    x_in = nc.dram_tensor("x_in", [B_SH, NIB, 128, L], F32, kind="ExternalInput")
    out_t = nc.dram_tensor("out", [B_SH, OC, TO], F32, kind="ExternalOutput")

    kshard = nc.dram_tensor("kshard", [KD, NIB, 128, O_SH], BF16)
    kgath = nc.dram_tensor(
        "kgath", [NC, KD, NIB, 128, O_SH], BF16, addr_space="Shared"
    )

    with tile.TileContext(nc) as tc:
        # ---------------- phase 1: kernel construction ----------------
        with tc.tile_pool(name="smalls", bufs=1) as smalls, \
             tc.tile_pool(name="xbig", bufs=1) as xbig, \
             tc.tile_pool(name="dtmp", bufs=2) as dtmp:
            p_sb = smalls.tile([128, FB], F32)
            sig_sb = smalls.tile([128, FB], F32)
            w_sb = smalls.tile([128, FB], F32)
            sgn_sb = smalls.tile([128, FB], F32)
            nc.sync.dma_start(p_sb[:], p_in[:])
            nc.sync.dma_start(sig_sb[:], sig_in[:])
            nc.sync.dma_start(w_sb[:], w_in[:])
            nc.sync.dma_start(sgn_sb[:], sgn_in[:])

            # Pc = P + KD//2 (in place); R = 1/(|SIG| + 0.27) (in place)
            pc_sb = p_sb
            nc.vector.tensor_scalar_add(pc_sb[:], p_sb[:], float(KD // 2))
            nc.scalar.activation(sig_sb[:], sig_sb[:], AF.Abs)
            nc.vector.tensor_scalar_add(sig_sb[:], sig_sb[:], 0.27)
            rscr = smalls.tile([128, FB], F32)
            r_sb = sig_sb
            nc.vector.reciprocal_approx_accurate(r_sb[:], sig_sb[:], rscr[:])

            # Wp = weight * sign (in place into w_sb; Wn needs Z)
            wp_sb = w_sb
            nc.vector.tensor_mul(wp_sb[:], w_sb[:], sgn_sb[:])

            # X_d = exp(-0.5 * ((Pc - d) * R)^2), stored bf16, free = (d, j, c)
            x_all = xbig.tile([128, KD * FB], BF16)
            for d in range(KD):
                m = dtmp.tile([128, FB], F32, tag="m")
                nc.vector.scalar_tensor_tensor(
                    m[:], pc_sb[:], float(d), r_sb[:],
                    op0=ALU.subtract, op1=ALU.mult,
                )
                nc.scalar.activation(m[:], m[:], AF.Square)
                nc.scalar.activation(
                    x_all[:, d * FB:(d + 1) * FB], m[:], AF.Exp, scale=-0.5
                )

            # Z = sum_d X_d : bf16 pair adds (2x mode) + fp32 accumulation
            z_sb = smalls.tile([128, FB], F32)
            with nc.allow_low_precision("bf16 pair partials"):
                nc.vector.tensor_add(
                    z_sb[:], x_all[:, 0:FB], x_all[:, FB:2 * FB]
                )
                for i in range(1, KD // 2):
                    pr = dtmp.tile([128, FB], BF16, tag="pr")
                    nc.vector.tensor_add(
                        pr[:],
                        x_all[:, (2 * i) * FB:(2 * i + 1) * FB],
                        x_all[:, (2 * i + 1) * FB:(2 * i + 2) * FB],
                    )
                    nc.vector.tensor_add(z_sb[:], z_sb[:], pr[:])
                nc.vector.tensor_add(
                    z_sb[:], z_sb[:], x_all[:, (KD - 1) * FB:KD * FB]
                )

            # Wn = Wp / (Z + 1e-7), cast bf16 for the 2x-mode multiply
            nc.vector.tensor_scalar_add(z_sb[:], z_sb[:], 1e-7)
            nc.vector.reciprocal_approx_accurate(z_sb[:], z_sb[:], rscr[:])
            wn_sb = sgn_sb
            nc.vector.tensor_mul(wn_sb[:], wp_sb[:], z_sb[:])
            wn16 = smalls.tile([128, FB], BF16)
            nc.vector.tensor_copy(wn16[:], wn_sb[:])

            # K[p, (d, j)] = sum_c X_d[p, (j, c)] * Wn[p, (j, c)]
            k_sb = smalls.tile([128, KD * NT], BF16)
            with nc.allow_low_precision("bf16 conv weights"):
                for d in range(KD):
                    y = dtmp.tile([128, FB], BF16, tag="y")
                    nc.vector.tensor_mul(
                        y[:], x_all[:, d * FB:(d + 1) * FB], wn16[:]
                    )
                    y3 = y.rearrange("p (j c) -> p j c", c=KC)
                    nc.vector.reduce_sum(
                        k_sb[:, d * NT:(d + 1) * NT], y3,
                        axis=mybir.AxisListType.X,
                    )

            # store shard: k_sb [p, (d, ih, ol)] -> kshard (d, ih, p, ol)
            ksb_v = k_sb.rearrange("p (d ih ol) -> p d ih ol", d=KD, ih=NIB)
            kout_v = kshard[:].rearrange("d ih p ol -> p d ih ol")
            nc.sync.dma_start(kout_v, ksb_v)

        # ---------------- phase 2: all-gather ----------------
        nc.gpsimd.collective_compute(
            "AllGather",
            ALU.bypass,
            replica_groups=[list(range(NC))],
            ins=[kshard[:]],
            outs=[kgath[:]],
        )

        # ---------------- phase 3: conv ----------------
        with tc.tile_pool(name="kw", bufs=1) as kw, \
             tc.tile_pool(name="xp", bufs=1) as xp, \
             tc.tile_pool(name="ps", bufs=4, space="PSUM") as ps, \
             tc.tile_pool(name="ob", bufs=3) as obp:
            lhsT = {}
            for d in range(KD):
                for ih in range(NIB):
                    t = kw.tile([128, OC], BF16, tag=f"k{d}_{ih}")
                    src = kgath[:, d, ih, :, :].rearrange("core p ol -> p core ol")
                    nc.sync.dma_start(t[:].rearrange("p (core ol) -> p core ol", core=NC), src)
                    lhsT[(d, ih)] = t

            x_sb = {}
            for b in range(B_SH):
                for ih in range(NIB):
                    t = xp.tile([128, L], BF16, tag=f"x{b}_{ih}")
                    # casting DMA (f32 -> bf16) runs on the software DGE
                    nc.gpsimd.dma_start(t[:], x_in[b, ih, :, :])
                    x_sb[(b, ih)] = t

            for b in range(B_SH):
                for ob in range(NOB):
                    for tck in range(NTC):
                        acc = ps.tile([128, TC], F32, tag="acc")
                        n = 0
                        for ih in range(NIB):
                            for d in range(KD):
                                nc.tensor.matmul(
                                    acc[:],
                                    lhsT[(d, ih)][:, ob * 128:(ob + 1) * 128],
                                    x_sb[(b, ih)][:, tck * TC + d: tck * TC + d + TC],
                                    start=(n == 0),
                                    stop=(n == NIB * KD - 1),
                                )
                                n += 1
                        o_sb = obp.tile([128, TC], F32, tag="osb")
                        nc.scalar.copy(o_sb[:], acc[:])
                        nc.sync.dma_start(
                            out_t[b, ob * 128:(ob + 1) * 128, tck * TC:(tck + 1) * TC],
                            o_sb[:],
                        )

    nc.compile()
    return nc


def make_in_maps(x, weight, sign, P, SIG):
    """Slice/pack full inputs into per-core input maps (pure layout work)."""
    x = np.ascontiguousarray(x, dtype=np.float32)
    in_maps = []
    for c in range(NC):
        osl = slice(O_SH * c, O_SH * c + O_SH)

        def pack(a):
            # (O_SH, IC, KC) -> [p = i mod 128, (j = ih*32 + o_loc, c)]
            a = np.asarray(a, dtype=np.float32).reshape(O_SH, NIB, 128, KC)
            a = a.transpose(1, 0, 2, 3).reshape(NT, 128, KC)
            return np.ascontiguousarray(
                a.transpose(1, 0, 2).reshape(128, NT * KC)
            )

        in_maps.append({
            "p_in": pack(P[0][osl]),
            "sig_in": pack(SIG[0][osl]),
            "w_in": pack(weight[osl]),
            "sgn_in": pack(sign[osl]),
            "x_in": np.ascontiguousarray(
                x[B_SH * c: B_SH * c + B_SH].reshape(B_SH, NIB, 128, L)
            ),
        })
    return in_maps


_CACHED = {}


def kernel(x, weight, sign, P, SIG, trace=False):
    if "nc" not in _CACHED:
        _CACHED["nc"] = build_module()
    nc = _CACHED["nc"]
    in_maps = make_in_maps(x, weight, sign, P, SIG)
    res = run_bass_kernel_spmd(
        nc, in_maps, core_ids=list(range(NC)), trace=trace,
    )
    out = np.concatenate([r["out"] for r in res.results], axis=0)
    if trace:
        _CACHED["last_result"] = res
    return out


# revision 19
# speedup vs baseline: 1.3620x; 1.1677x over previous
"""Dcls1d (Gaussian-parameterized dilated conv1d) Trainium2 Bass kernel.

Math (reference):
    W   = weight * sign                               (O, I, C)
    Pc  = P[0] + KD//2 ; S = |SIG[0]| + 0.27          (O, I, C)
    X_d = exp(-0.5 * ((d - Pc)/S)^2)                  d = 0..KD-1
    K   = sum_c X_d * W / (sum_d' X_d' + 1e-7)        (O, I, KD)
    out = conv1d(x, K, VALID)                         (B, O, L-KD+1)

Distribution over 8 NeuronCores:
  - kernel construction: out-channel-sharded (32 out-channels per core)
  - AllGather of the (O, I, KD) kernel (as matmul-ready lhsT layout)
  - conv: batch-sharded (4 batches per core), 50 accumulating PE matmuls
    per output tile in float32r (single-pass fp22).

Construction layout per core: partitions p = i mod 128, free = (j, c) where
j = ih*32 + o_loc indexes the 64 (o_loc, i-half) tiles. The per-d Gaussian
argument is one fused scalar_tensor_tensor: m = (Pc - d) * R, whose sign is
killed by Square. X is stored bf16; Z is one strided reduce over d.
"""

import os

import numpy as np

import concourse.bass as bass
import concourse.mybir as mybir
import concourse.tile as tile
from concourse import bacc
from concourse.bass_utils import run_bass_kernel_spmd

F32 = mybir.dt.float32
F32R = mybir.dt.float32r
BF16 = mybir.dt.bfloat16
AF = mybir.ActivationFunctionType
ALU = mybir.AluOpType

B, OC, IC, L = 32, 256, 256, 1024
KC, KD = 26, 25
NC = 8
O_SH = OC // NC          # 32 out-channels per core
NIB = IC // 128          # 2 i-blocks
NT = O_SH * NIB          # 64 construction tiles per core
FB = NT * KC             # 1664 free width of the "small" tensors
B_SH = B // NC           # 4 batches per core
TO = L - KD + 1          # 1000 output positions
TC = 500                 # conv t-chunk (PSUM bank = 512 fp32 max)
NTC = TO // TC           # 2
NOB = OC // 128          # 2 out-channel blocks


def build_module():
    nc = bacc.Bacc("TRN2", num_devices=NC)

    p_in = nc.dram_tensor("p_in", [128, FB], F32, kind="ExternalInput")
    sig_in = nc.dram_tensor("sig_in", [128, FB], F32, kind="ExternalInput")
    w_in = nc.dram_tensor("w_in", [128, FB], F32, kind="ExternalInput")
    sgn_in = nc.dram_tensor("sgn_in", [128, FB], F32, kind="ExternalInput")
    x_in = nc.dram_tensor("x_in", [B_SH, NIB, 128, L], F32, kind="ExternalInput")
    out_t = nc.dram_tensor("out", [B_SH, OC, TO], F32, kind="ExternalOutput")

    # gather in two chunks so the first overlaps the tail of construction
    DCH = [(0, 13), (13, KD)]  # [d0, d1) ranges
    kshard = [
        nc.dram_tensor(f"kshard{i}", [d1 - d0, NIB, 128, O_SH], BF16)
        for i, (d0, d1) in enumerate(DCH)
    ]
    kgath = [
        nc.dram_tensor(
            f"kgath{i}", [NC, d1 - d0, NIB, 128, O_SH], BF16, addr_space="Shared"
        )
        for i, (d0, d1) in enumerate(DCH)
    ]

    with tile.TileContext(nc) as tc:
        # ---------------- phase 1: kernel construction ----------------
        with tc.tile_pool(name="smalls", bufs=1) as smalls, \
             tc.tile_pool(name="xbig", bufs=1) as xbig, \
             tc.tile_pool(name="dtmp", bufs=2) as dtmp:
            p_sb = smalls.tile([128, FB], F32)
            sig_sb = smalls.tile([128, FB], F32)
            w_sb = smalls.tile([128, FB], F32)
            sgn_sb = smalls.tile([128, FB], F32)
            nc.sync.dma_start(p_sb[:], p_in[:])
            nc.sync.dma_start(sig_sb[:], sig_in[:])
            nc.sync.dma_start(w_sb[:], w_in[:])
            nc.sync.dma_start(sgn_sb[:], sgn_in[:])

            # Pc = P + KD//2 (in place); R = 1/(|SIG| + 0.27) (in place)
            pc_sb = p_sb
            nc.vector.tensor_scalar_add(pc_sb[:], p_sb[:], float(KD // 2))
            nc.scalar.activation(sig_sb[:], sig_sb[:], AF.Abs)
            nc.vector.tensor_scalar_add(sig_sb[:], sig_sb[:], 0.27)
            rscr = smalls.tile([128, FB], F32)
            r_sb = sig_sb
            nc.vector.reciprocal_approx_accurate(r_sb[:], sig_sb[:], rscr[:])

            # Wp = weight * sign (in place into w_sb; Wn needs Z)
            wp_sb = w_sb
            nc.vector.tensor_mul(wp_sb[:], w_sb[:], sgn_sb[:])

            # X_d = exp(-0.5 * ((Pc - d) * R)^2), stored bf16, free = (d, j, c)
            x_all = xbig.tile([128, KD * FB], BF16)
            # X'_d = c * exp(-0.5*((Pc-d)*R)^2):
            #   HW path: one ACT op via erf'(m/sqrt(2)) = (2/sqrt(pi))*exp(-m^2/2)
            #   (c = 2/sqrt(pi), folded into eps below). Sim path: Square+Exp
            #   (Derivative_Erf is not implemented by the simulator); c = 1.
            use_derf = os.environ.get("DCLS_SIM_EXP", "0") != "1"
            c_gauss = 1.1283791670955126 if use_derf else 1.0
            for d in range(KD):
                m = dtmp.tile([128, FB], F32, tag="m")
                nc.vector.scalar_tensor_tensor(
                    m[:], pc_sb[:], float(d), r_sb[:],
                    op0=ALU.subtract, op1=ALU.mult,
                )
                if use_derf:
                    nc.scalar.activation(
                        x_all[:, d * FB:(d + 1) * FB], m[:], AF.Derivative_Erf,
                        scale=0.7071067811865476,
                    )
                else:
                    nc.scalar.activation(m[:], m[:], AF.Square)
                    nc.scalar.activation(
                        x_all[:, d * FB:(d + 1) * FB], m[:], AF.Exp, scale=-0.5
                    )

            # Z = sum_d X_d : bf16 4-way groups + tree (TT adds in 2x mode),
            # manual slices of one buffer; DVE program order resolves WARs.
            zbuf = smalls.tile([128, 8 * FB], BF16)
            zs = [zbuf[:, i * FB:(i + 1) * FB] for i in range(8)]
            xs = [x_all[:, d * FB:(d + 1) * FB] for d in range(KD)]
            z_sb = smalls.tile([128, FB], F32)
            with nc.allow_low_precision("bf16 partial sums"):
                for g in range(6):
                    nc.vector.tensor_add(zs[6], xs[4 * g], xs[4 * g + 1])
                    nc.vector.tensor_add(zs[7], xs[4 * g + 2], xs[4 * g + 3])
                    nc.vector.tensor_add(zs[g], zs[6], zs[7])
                nc.vector.tensor_add(zs[6], zs[0], zs[1])
                nc.vector.tensor_add(zs[7], zs[2], zs[3])
                nc.vector.tensor_add(zs[0], zs[4], zs[5])
                nc.vector.tensor_add(zs[1], zs[6], zs[7])
                nc.vector.tensor_add(zs[2], zs[1], zs[0])
                nc.vector.tensor_add(z_sb[:], zs[2], xs[KD - 1])

            # Wn = Wp / (Z + c*1e-7), cast bf16 for the 2x-mode multiply
            nc.vector.tensor_scalar_add(z_sb[:], z_sb[:], c_gauss * 1e-7)
            nc.vector.reciprocal_approx_accurate(z_sb[:], z_sb[:], rscr[:])
            wn_sb = sgn_sb
            nc.vector.tensor_mul(wn_sb[:], wp_sb[:], z_sb[:])
            wn16 = smalls.tile([128, FB], BF16)
            nc.vector.tensor_copy(wn16[:], wn_sb[:])

            # K[p, (d, j)] = sum_c X_d[p, (j, c)] * Wn[p, (j, c)]
            k_sb = smalls.tile([128, KD * NT], BF16)
            with nc.allow_low_precision("bf16 conv weights"):
                for d in range(KD):
                    y = dtmp.tile([128, FB], BF16, tag="y")
                    nc.vector.tensor_mul(
                        y[:], x_all[:, d * FB:(d + 1) * FB], wn16[:]
                    )
                    y3 = y.rearrange("p (j c) -> p j c", c=KC)
                    nc.vector.reduce_sum(
                        k_sb[:, d * NT:(d + 1) * NT], y3,
                        axis=mybir.AxisListType.X,
                    )

            # store shard chunks: k_sb [p, (d, ih, ol)] -> kshard (d, ih, p, ol)
            for i, (d0, d1) in enumerate(DCH):
                ksb_v = k_sb[:, d0 * NT:d1 * NT].rearrange(
                    "p (d ih ol) -> p d ih ol", ih=NIB, ol=O_SH
                )
                kout_v = kshard[i][:].rearrange("d ih p ol -> p d ih ol")
                nc.sync.dma_start(kout_v, ksb_v)

        # ---------------- phase 2: all-gather (chunked) ----------------
        for i in range(len(DCH)):
            nc.gpsimd.collective_compute(
                "AllGather",
                ALU.bypass,
                replica_groups=[list(range(NC))],
                ins=[kshard[i][:]],
                outs=[kgath[i][:]],
            )

        # ---------------- phase 3: conv ----------------
        with tc.tile_pool(name="kw", bufs=1) as kw, \
             tc.tile_pool(name="xp", bufs=1) as xp, \
             tc.tile_pool(name="ps", bufs=4, space="PSUM") as ps, \
             tc.tile_pool(name="ob", bufs=3) as obp:
            lhsT = {}
            for d in range(KD):
                ci = 0 if d < DCH[0][1] else 1
                dl = d - DCH[ci][0]
                for ih in range(NIB):
                    t = kw.tile([128, OC], BF16, tag=f"k{d}_{ih}")
                    src = kgath[ci][:, dl, ih, :, :].rearrange("core p ol -> p core ol")
                    nc.sync.dma_start(t[:].rearrange("p (core ol) -> p core ol", core=NC), src)
                    lhsT[(d, ih)] = t

            x_sb = {}
            for b in range(B_SH):
                for ih in range(NIB):
                    t = xp.tile([128, L], BF16, tag=f"x{b}_{ih}")
                    # casting DMA (f32 -> bf16) runs on the software DGE
                    nc.gpsimd.dma_start(t[:], x_in[b, ih, :, :])
                    x_sb[(b, ih)] = t

            for b in range(B_SH):
                for ob in range(NOB):
                    for tck in range(NTC):
                        acc = ps.tile([128, TC], F32, tag="acc")
                        n = 0
                        for ih in range(NIB):
                            for d in range(KD):
                                nc.tensor.matmul(
                                    acc[:],
                                    lhsT[(d, ih)][:, ob * 128:(ob + 1) * 128],
                                    x_sb[(b, ih)][:, tck * TC + d: tck * TC + d + TC],
                                    start=(n == 0),
                                    stop=(n == NIB * KD - 1),
                                )
                                n += 1
                        o_sb = obp.tile([128, TC], F32, tag="osb")
                        nc.scalar.copy(o_sb[:], acc[:])
                        nc.sync.dma_start(
                            out_t[b, ob * 128:(ob + 1) * 128, tck * TC:(tck + 1) * TC],
                            o_sb[:],
                        )

    nc.compile()
    return nc


def make_in_maps(x, weight, sign, P, SIG):
    """Slice/pack full inputs into per-core input maps (pure layout work)."""
    x = np.ascontiguousarray(x, dtype=np.float32)
    in_maps = []
    for c in range(NC):
        osl = slice(O_SH * c, O_SH * c + O_SH)

        def pack(a):
            # (O_SH, IC, KC) -> [p = i mod 128, (j = ih*32 + o_loc, c)]
            a = np.asarray(a, dtype=np.float32).reshape(O_SH, NIB, 128, KC)
            a = a.transpose(1, 0, 2, 3).reshape(NT, 128, KC)
            return np.ascontiguousarray(
                a.transpose(1, 0, 2).reshape(128, NT * KC)
            )

        in_maps.append({
            "p_in": pack(P[0][osl]),
            "sig_in": pack(SIG[0][osl]),
            "w_in": pack(weight[osl]),
            "sgn_in": pack(sign[osl]),
            "x_in": np.ascontiguousarray(
                x[B_SH * c: B_SH * c + B_SH].reshape(B_SH, NIB, 128, L)
            ),
        })
    return in_maps


_CACHED = {}


def kernel(x, weight, sign, P, SIG, trace=False):
    if "nc" not in _CACHED:
        _CACHED["nc"] = build_module()
    nc = _CACHED["nc"]
    in_maps = make_in_maps(x, weight, sign, P, SIG)
    res = run_bass_kernel_spmd(
        nc, in_maps, core_ids=list(range(NC)), trace=trace,
    )
    out = np.concatenate([r["out"] for r in res.results], axis=0)
    if trace:
        _CACHED["last_result"] = res
    return out
